# revision 79
# baseline (speedup 1.0000x reference)
"""Trainium2 Bass kernel for a multi-head ReLU-attention transformer layer.

Shapes (hardcoded): B=32, F=1024, DIN=64, DOUT=64, H=4.
  qkv   = einsum("bfi,hkio->bhkfo", x, Wqkv)
  scores= relu(q @ k^T / sqrt(DOUT))
  head  = scores @ v
  out   = LN(concat(head) @ Wo + bo + x) * gamma + beta

Sharding: pure data-parallel over batch B across 8 NeuronCores (4 b/core).

Host-side algebraic folds (exact or fp32-precise):
  - Wk folded into Wq:  scores_h = x @ A_h @ x^T with A_h = Wq_h Wk_h^T / 8.
    Kills the K projection entirely (x^T serves as the score stationary).
  - Wo folded into Wv:  proj = sum_h scores_h @ (Wv_h @ Wo_h) = sum_h sc_h V'_h.

Per-batch device pipeline (all matmuls bf16 with fp32 PSUM accumulation —
fp32/fp32r matmuls silently return zeros on this toolchain):
  xT arrives from HBM pre-transposed/bf16-cast on the host (pure layout
  work), duplicated onto both partition halves so either PE row group can
  serve the 64-deep contraction; batch 0's first xt DMA carries the folded
  weights as a prefix (a separate weight DMA costs ~2.7us of fixed DMA
  latency on the critical path).  U^T = A^T x^T (head pairs stacked on M).
  Attention runs in two f-half passes.  scoresT_h = relu(xT_g^T @ U^T_h)
  drains PSUM->SBUF bf16 via ScalarE/VectorE (the bandwidth-critical path:
  PSUM fp32 reads are capped at 1 elem/lane/cycle and only ACT/DVE have
  PSUM ports; a greedy ns-accumulator balances the two queues).  The
  out-projection uses the drained scoresT as the matmul STATIONARY operand
  (stationary loads are pipelined behind compute, so each call costs only
  its N=64 moving columns -> 2x fewer PE columns than the moving-scores
  form), accumulating proj[f,o] for the pass's 4 f-tiles in one four-bank
  PSUM tile whose regions all START at bank boundaries (matmul PSUM writes
  at sub-bank offsets fail on this hardware).  proj lands in natural [f,o]
  layout: the residual add fuses with the PSUM drain, no transpose needed.
  LayerNorm in fp32; SBUF-only elementwise work rides on Pool (no PSUM
  port).  A global skid-2 deque defers each iteration's out-matmuls so the
  in-order PE never blocks on a score drain, and carries across pass/batch
  boundaries so the drain stream never dries up; each pass's LN half-tail
  is emitted in four pieces spread over the next pass's iterations so its
  DVE ops never queue ahead of critical score drains.

This walrus build accepts only ONE sync wait per instruction; Tile emits
multi-waits, so split_multiwaits() hoists extras onto NoOps post-schedule.
"""

import numpy as np

import concourse.bass as bass
import concourse.mybir as mybir
import concourse.tile as tile
from concourse.bass_utils import run_bass_kernel_spmd


def split_multiwaits(nc):
    """Hoist all but the last sync wait of any instruction onto standalone
    NoOps inserted just before it on the same engine — semantically identical
    (same-engine program order runs the waits first), but keeps every
    instruction within this walrus build's one-wait limit."""
    n_split = 0
    max_upd = 0

    def fix_block(bl):
        nonlocal n_split, max_upd
        insts = list(bl.instructions)
        out = []
        changed = False
        for inst in insts:
            si = inst.sync_info
            if si is not None:
                max_upd = max(max_upd, len(si.on_update))
                waits = list(si.on_wait)
                if len(waits) > 1:
                    for k, w in enumerate(waits[:-1]):
                        nop = mybir.InstNoOp(
                            name=f"{inst.name}-wsplit{k}", ins=[], outs=[])
                        nop.engine = inst.engine
                        nop.sync_info = mybir.SyncInfo(
                            on_wait=[w], on_update=[])
                        out.append(nop)
                    inst.sync_info = mybir.SyncInfo(
                        on_wait=[waits[-1]], on_update=list(si.on_update))
                    n_split += 1
                    changed = True
            out.append(inst)
        if changed:
            bl.instructions = out
        for sub in getattr(bl, "blocks", None) or []:
            fix_block(sub)

    for f in nc.m.functions:
        for bl in f.blocks:
            fix_block(bl)
    assert max_upd <= 1, f"need update-splitting too: {max_upd}"
    return n_split


B, F, DIN, DOUT, H = 32, 1024, 64, 64, 4
NCORES = 8
BPC = B // NCORES  # batches per core
NT = F // 128  # 8 f-tiles per batch
FP32 = mybir.dt.float32
BF16 = mybir.dt.bfloat16
EPS = 1e-5

_cache = {}


def _build(use_gb: bool, use_bo: bool, stage: int = 99):
    nc = bass.Bass("TRN2", target_bir_lowering=False, debug=False,
                   num_devices=NCORES)
    x_d = nc.dram_tensor("x", [BPC, F, DIN], FP32, kind="ExternalInput").ap()
    # xt rows carry [wa | wv | xT] per batch; the weights are read ONCE
    # from batch 0's prefix in the same DMA as its first xT half, so the
    # very first U matmul waits on a single DMA latency
    xt_d = nc.dram_tensor("xt", [BPC, 128, 384 + F], BF16,
                          kind="ExternalInput").ap()
    if use_gb:
        gb_d = nc.dram_tensor("gb", [2, DIN], FP32, kind="ExternalInput").ap()
    if use_bo:
        bo_d = nc.dram_tensor("bo", [DIN], FP32, kind="ExternalInput").ap()
    y_d = nc.dram_tensor("y", [BPC, F, DIN], FP32, kind="ExternalOutput").ap()

    # ACT/DVE drain balancing: greedy on accumulated engine-ns.  Per
    # [128,512] PSUM drain: ACT = 512 els/1.2GHz + init ~= 612 ns, DVE =
    # 512/0.96 + init ~= 658 ns (engines process 1 elem/lane/cycle from
    # PSUM regardless of dtype).
    # DVE starts pre-biased: it also carries the uncharged LN work
    # (res-add/reduce/reciprocal are DVE-only), so ACT takes a few extra
    # early drains (value tuned against TimelineSim)
    drain_load = [0.0, 1925.0]  # ACT, DVE accumulated ns
    ACT_NS, DVE_NS = 612.0, 658.0

    def pick_engine(pair=None):
        act = drain_load[0] + ACT_NS <= drain_load[1] + DVE_NS
        drain_load[0 if act else 1] += ACT_NS if act else DVE_NS
        return act

    def drain_relu(out_ap, in_ap, pair=None):
        if pick_engine(pair):
            nc.scalar.activation(out=out_ap, in_=in_ap,
                                 func=mybir.ActivationFunctionType.Relu)
        else:
            nc.vector.tensor_scalar_max(out=out_ap, in0=in_ap, scalar1=0.0)

    def drain_copy(out_ap, in_ap, pair=None):
        if pick_engine(pair):
            nc.scalar.activation(out=out_ap, in_=in_ap,
                                 func=mybir.ActivationFunctionType.Copy)
        else:
            nc.vector.tensor_copy(out=out_ap, in_=in_ap)

    with tile.TileContext(nc) as tc:
        with (
            tc.tile_pool(name="const", bufs=1) as constp,
            tc.tile_pool(name="xp", bufs=3) as xp,
            tc.tile_pool(name="xtp", bufs=3) as xtp,
            tc.tile_pool(name="utp", bufs=3) as utp,
            tc.tile_pool(name="vp", bufs=3) as vp,
            tc.tile_pool(name="scp", bufs=8) as scp,
            tc.tile_pool(name="resp", bufs=3) as resp,
            tc.tile_pool(name="statp", bufs=4) as statp,
            tc.tile_pool(name="mm", bufs=4, space="PSUM") as psmm,
            tc.tile_pool(name="acc", bufs=1, space="PSUM") as psacc,
        ):
            # ---- constants ----
            eps_sb = constp.tile([128, 1], FP32)
            nc.vector.memset(eps_sb, EPS)
            # weights land inside batch 0's first xt DMA (see below);
            # xt0 has bufs=1 and a unique tag so it is never recycled and
            # later batches keep reading the weight slices from it
            xt0 = xtp.tile([128, 384 + F], BF16, tag="xt0", bufs=1)
            wa_sb = xt0[:, 0:128]
            wv_sb = xt0[:, 128:384]
            if use_gb:
                g_rep = constp.tile([128, NT, DIN], FP32)
                b_rep = constp.tile([128, NT, DIN], FP32)
                for t in range(NT):
                    nc.gpsimd.dma_start(
                        out=g_rep[:, t, :],
                        in_=bass.AP(gb_d.tensor, 0, [[0, 128], [1, DIN]]))
                    nc.gpsimd.dma_start(
                        out=b_rep[:, t, :],
                        in_=bass.AP(gb_d.tensor, DIN, [[0, 128], [1, DIN]]))
            if use_bo:
                bo_rep = constp.tile([128, DIN], FP32)
                nc.gpsimd.dma_start(
                    out=bo_rep,
                    in_=bass.AP(bo_d.tensor, 0, [[0, 128], [1, DIN]]))

            # each batch's LN tail is emitted 3 iterations into the NEXT
            # batch's attention loop: its DVE-only ops (res-add, reduces)
            # then queue BEHIND the next batch's critical early score drains
            # instead of ahead of them (the in-order DVE queue otherwise
            # stalls PE via PSUM-tile rotation)
            deferred_tail = [None]
            pending = []

            for b in range(BPC):
                # ---- load xT (host pre-transposed bf16, already duplicated
                # onto both partition halves) in two halves so the first U/V
                # matmuls start after ~one half-DMA of latency; x fp32 loads
                # too (residual only — off the critical path) ----
                if b == 0:
                    xt_full, xo = xt0, 384
                else:
                    xt_full = xtp.tile([128, F], BF16, tag="xt",
                                       name=f"xt_{b}")
                    xo = 0
                for lh in range(2):
                    pre = 384 if (b == 0 and lh == 0) else 0
                    nc.sync.dma_start(
                        out=xt_full[:, xo + lh * (F // 2) - pre:
                                    xo + (lh + 1) * (F // 2)],
                        in_=xt_d[b][:, 384 + lh * (F // 2) - pre:
                                    384 + (lh + 1) * (F // 2)])
                xt = xt_full[:, xo:xo + F]
                x_sb = xp.tile([128, NT, DIN], FP32, tag="x")
                nc.sync.dma_start(
                    out=x_sb, in_=x_d[b].rearrange("(t p) j -> p t j", p=128))
                if use_bo:
                    x_res = xp.tile([128, NT, DIN], FP32, tag="xres")
                    for t in range(NT):
                        nc.vector.tensor_add(
                            out=x_res[:, t, :], in0=x_sb[:, t, :], in1=bo_rep)
                else:
                    x_res = x_sb

                if stage < 2:
                    nc.sync.dma_start(
                        out=y_d[b].rearrange("(t p) j -> p t j", p=128),
                        in_=x_sb)
                    continue
                # ---- U^T = A^T x^T (heads 2hp,2hp+1 stacked on M) ----
                ut = []
                for hp in range(2):
                    psl = bass.ds(64 * hp, 64)
                    u_sb = utp.tile([128, F], BF16, tag=f"ut{hp}")
                    for uc in range(2):
                        u_ps = psmm.tile([128, 512], FP32, tag="mm",
                                         name=f"u_ps{hp}{uc}_{b}")
                        nc.tensor.matmul(
                            u_ps, wa_sb[psl, :],
                            xt[psl, bass.ts(uc, 512)], start=True, stop=True)
                        drain_copy(u_sb[:, bass.ts(uc, 512)], u_ps)
                    ut.append(u_sb)

                if stage < 3:
                    nc.sync.dma_start(
                        out=y_d[b].rearrange("(t p) j -> p t j", p=128),
                        in_=x_sb)
                    continue
                # v' = x @ (Wv@Wo): natural [g, (h o)=256].  Matmul PSUM
                # outputs must START at a bank boundary on this hardware, so
                # two g-tiles share a two-bank tile at offsets 0 and 512 and
                # one strided drain picks up both [*,0:256] halves.
                vt = vp.tile([128, NT, 256], BF16, tag="v")
                for gt in range(NT):
                    v_ps = psmm.tile([128, 512], FP32, tag="mm",
                                     name=f"v_ps{gt}_{b}")
                    hsl = bass.ds(64 * (gt % 2), 64)
                    nc.tensor.matmul(
                        v_ps[:, 0:256],
                        xt[hsl, bass.ts(gt, 128)],
                        wv_sb[hsl, :],
                        start=True, stop=True)
                    drain_copy(vt[:, gt, :], v_ps[:, 0:256])

                if stage < 4:
                    nc.sync.dma_start(
                        out=y_d[b].rearrange("(t p) j -> p t j", p=128),
                        in_=x_sb)
                    continue
                # ---- attention in two fc passes (512 f-columns each).
                # Per pass the proj accumulator is ONE [128,2048] four-bank
                # tile whose per-f-tile regions [:, 512*tw : 512*tw+64] all
                # START at bank boundaries (hardware requires bank-aligned
                # matmul outputs).  Scores for both heads of a pair share a
                # [128,1024] tile at offsets 0/512 (also bank starts) and
                # drain in one [128,1024] read.  The drained scoresT is the
                # out-matmul STATIONARY operand (N=64 moving columns -> 2x
                # fewer PE columns than the moving-scores form) and proj
                # lands in natural [f, o] layout: the residual add fuses
                # with the PSUM drain and no transpose is ever needed.
                # Each pass's f-half LN tail overlaps the next pass. ----
                out_ps = psacc.tile([128, 4, 512], FP32, tag="acc",
                                    name=f"out_ps_{b}")

                NH = NT // 2
                for fc in range(2):
                    started = [False] * 4

                    def emit_out_mms(gt, pair, last, started=started,
                                     out_ps=out_ps, vt=vt):
                        for j, h, sc_sb in pair:
                            for tw in range(4):
                                nc.tensor.matmul(
                                    out_ps[:, tw, 0:64],
                                    sc_sb[:, bass.ds(512 * j + 128 * tw, 128)],
                                    vt[:, gt, bass.ds(64 * h, 64)],
                                    start=not started[tw],
                                    stop=last and j == 1 and h == 3,
                                    skip_group_check=True)
                                started[tw] = True

                    # software pipeline: defer each gt's out-MMs TWO
                    # iterations so the in-order PE never head-of-line
                    # blocks on a score drain.  The deque is GLOBAL: it
                    # carries across pass and batch boundaries, so the final
                    # out-MM flush of one pass interleaves with the next
                    # pass's score matmuls and the drain stream never dries
                    # up (otherwise ACT/DVE bubble at every pass boundary).
                    for hp in range(2):
                        for gt in range(NT):
                            gsl = bass.ts(gt, 128)
                            sc_sb = scp.tile([128, 1024], BF16, tag="sc",
                                             name=f"sc_{b}_{fc}_{hp}_{gt}")
                            pair = []
                            for j in range(2):
                                h = 2 * hp + j
                                hsl = bass.ds(64 * j, 64)
                                sc_ps = psmm.tile(
                                    [128, 512], FP32, tag="mm",
                                    name=f"s_{b}_{fc}_{hp}_{gt}_{j}")
                                nc.tensor.matmul(
                                    sc_ps,
                                    xt[hsl, gsl],
                                    ut[hp][hsl, bass.ds(512 * fc, 512)],
                                    start=True, stop=True)
                                drain_relu(sc_sb[:, bass.ts(j, 512)], sc_ps,
                                           pair=j)
                                pair.append((j, h, sc_sb))
                            pending.append(
                                (emit_out_mms,
                                 (gt, pair, hp == 1 and gt == NT - 1)))
                            # deferred half-tail pieces: piece 0 (the
                            # out_ps-reading res-add) must be emitted after
                            # the PREVIOUS pass's final out-MMs (popped at
                            # gt 0 and 1) and before THIS pass's first
                            # region-clearing out-MM (popped at gt 2);
                            # later pieces go every 4th iteration
                            it = hp * NT + gt
                            if (deferred_tail[0] and it >= 2
                                    and (it - 2) % 4 == 0):
                                deferred_tail[0].pop(0)()
                                if not deferred_tail[0]:
                                    deferred_tail[0] = None
                            if len(pending) > 2:
                                fn, args = pending.pop(0)
                                fn(*args)

                    # ---- half tail: fused drain+residual (natural layout,
                    # strided read over the 4 region banks) then LayerNorm.
                    # SBUF-only elementwise work rides on Pool (no PSUM
                    # port).  Emitted as FOUR pieces spread over the next
                    # pass's iterations so the DVE queue never takes a large
                    # contiguous LN block ahead of that pass's score drains
                    # (which would stall PE via PSUM-tile rotation). ----
                    def make_tail(t0, ntl, b=b, fc=fc, out_ps=out_ps,
                                  x_res=x_res,
                                  last=(b == BPC - 1 and fc == 1)):
                        tsl = slice(t0, t0 + ntl)
                        r0 = t0 - fc * NH
                        rsl = slice(r0, r0 + ntl)
                        res = resp.tile([128, ntl, DIN], FP32,
                                        tag=f"res{fc}{r0}",
                                        name=f"res{fc}{r0}_{b}")
                        sq = resp.tile([128, ntl, DIN], FP32,
                                       tag=f"sq{fc}{r0}",
                                       name=f"sq{fc}{r0}_{b}")
                        stat = statp.tile([128, ntl, 2], FP32,
                                          tag=f"st{fc}{r0}",
                                          name=f"st{fc}{r0}_{b}")
                        mv = statp.tile([128, ntl, 4], FP32,
                                        tag=f"mv{fc}{r0}",
                                        name=f"mv{fc}{r0}_{b}")
                        o_sb = resp.tile([128, ntl, DIN], FP32,
                                         tag=f"o{fc}{r0}",
                                         name=f"o{fc}{r0}_{b}")
                        # terminal tail: DVE is idle and its ops are ~2x
                        # lower-latency than Pool's (no Q7 launch)
                        ln = nc.vector if last else nc.gpsimd

                        def p0():
                            nc.vector.tensor_add(
                                out=res,
                                in0=out_ps[:, rsl, 0:64],
                                in1=x_res[:, tsl, :])
                            ln.tensor_mul(out=sq, in0=res, in1=res)

                        def p1():
                            nc.vector.tensor_reduce(
                                out=stat[:, :, 0], in_=res,
                                axis=mybir.AxisListType.X,
                                op=mybir.AluOpType.add)

                        def p2():
                            nc.vector.tensor_reduce(
                                out=stat[:, :, 1], in_=sq,
                                axis=mybir.AxisListType.X,
                                op=mybir.AluOpType.add)
                            # mean, E[x^2]
                            ln.tensor_scalar_mul(
                                out=mv[:, :, 0], in0=stat[:, :, 0],
                                scalar1=1.0 / DIN)
                            ln.tensor_scalar_mul(
                                out=mv[:, :, 1], in0=stat[:, :, 1],
                                scalar1=1.0 / DIN)
                            # var = E[x^2] - mean^2
                            ln.tensor_mul(
                                out=mv[:, :, 2], in0=mv[:, :, 0],
                                in1=mv[:, :, 0])
                            ln.tensor_sub(
                                out=mv[:, :, 2], in0=mv[:, :, 1],
                                in1=mv[:, :, 2])
                            # rstd = 1/sqrt(var + eps)
                            nc.scalar.activation(
                                out=mv[:, :, 3], in_=mv[:, :, 2],
                                func=mybir.ActivationFunctionType.Sqrt,
                                bias=eps_sb)

                        def p3():
                            nc.vector.reciprocal(
                                out=mv[:, :, 3], in_=mv[:, :, 3])
                            for k in range(ntl):
                                ln.tensor_scalar(
                                    out=o_sb[:, k, :], in0=res[:, k, :],
                                    scalar1=mv[:, k, 0:1],
                                    scalar2=mv[:, k, 3:4],
                                    op0=mybir.AluOpType.subtract,
                                    op1=mybir.AluOpType.mult)
                            if use_gb:
                                ln.tensor_mul(
                                    out=o_sb, in0=o_sb, in1=g_rep[:, tsl, :])
                                ln.tensor_add(
                                    out=o_sb, in0=o_sb, in1=b_rep[:, tsl, :])
                            # y-store issued from the ACT sequencer (HWDGE):
                            # keeps the in-order SP queue free for the next
                            # batch's x-load, Pool free of SWDGE desc-gen
                            y_nat = y_d[b].rearrange(
                                "(t p) j -> p t j", p=128)
                            if last:
                                # terminal: store in two quarters so the
                                # first transfer overlaps the last applies;
                                # the gating second quarter rides the Pool
                                # SWDGE queue (idle at kernel end, shorter
                                # total path than ACT HWDGE; Pool y-stores
                                # are hardware-proven, unlike SP which
                                # wedges the device at kernel end)
                                nc.scalar.dma_start(
                                    out=y_nat[:, t0:t0 + 2, :],
                                    in_=o_sb[:, 0:2, :])
                                nc.gpsimd.dma_start(
                                    out=y_nat[:, t0 + 2:t0 + 4, :],
                                    in_=o_sb[:, 2:4, :])
                            else:
                                nc.scalar.dma_start(
                                    out=y_nat[:, tsl, :], in_=o_sb)

                        return [p0, p1, p2, p3]

                    if b == BPC - 1 and fc == 1:
                        for fn, args in pending:
                            fn(*args)
                        pending.clear()
                        for p in make_tail(fc * NH, NH):
                            p()
                    else:
                        # every piece of the previous tail must have fired
                        # (a leftover piece would be silently dropped here,
                        # losing that half's LN + y-store)
                        assert deferred_tail[0] is None, "tail piece dropped"
                        deferred_tail[0] = make_tail(fc * NH, NH)




    split_multiwaits(nc)
    return nc


def kernel(featureVec, Wqkv, Wo, bo, ln_gamma, ln_beta):
    x = np.ascontiguousarray(np.asarray(featureVec, dtype=np.float32))
    Wqkv = np.asarray(Wqkv, dtype=np.float32)
    Wo = np.asarray(Wo, dtype=np.float32)
    bo = np.asarray(bo, dtype=np.float32)
    g = np.asarray(ln_gamma, dtype=np.float32)
    be = np.asarray(ln_beta, dtype=np.float32)

    # host-side weight folding:  A_h = Wq_h Wk_h^T / 8,  V'_h = Wv_h Wo_h
    a_pack = np.concatenate(
        [(Wqkv[h, 0].astype(np.float64)
          @ Wqkv[h, 1].astype(np.float64).T * 0.125).astype(np.float32)
         for h in range(H)], axis=1)  # [64, 256]
    wv_pack = np.concatenate(
        [(Wqkv[h, 2].astype(np.float64)
          @ Wo[h * DOUT:(h + 1) * DOUT].astype(np.float64)).astype(np.float32)
         for h in range(H)], axis=1)  # [64, 256]
    import ml_dtypes
    bf = ml_dtypes.bfloat16
    wa_host = np.ascontiguousarray(
        np.concatenate([a_pack[:, 0:128], a_pack[:, 128:256]],
                       axis=0).astype(bf))  # [128, 128]
    wv_host = np.ascontiguousarray(
        np.concatenate([wv_pack, wv_pack], axis=0).astype(bf))  # [128, 256]
    # xT per batch, bf16, duplicated onto both partition halves, prefixed
    # with the folded weights [wa | wv]: [B, 128, 384 + F]
    xt_half = np.transpose(x, (0, 2, 1)).astype(bf)  # [B, 64, F]
    xt_dup = np.concatenate([xt_half, xt_half], axis=1)  # [B, 128, F]
    w_blk = np.broadcast_to(
        np.concatenate([wa_host, wv_host], axis=1), (B, 128, 384))
    xt_host = np.ascontiguousarray(
        np.concatenate([w_blk, xt_dup], axis=2))  # [B, 128, 384 + F]

    use_gb = not (np.all(g == 1.0) and np.all(be == 0.0))
    use_bo = not np.all(bo == 0.0)

    key = (use_gb, use_bo)
    if key not in _cache:
        _cache[key] = _build(use_gb, use_bo)
    nc = _cache[key]

    in_maps = []
    for c in range(NCORES):
        m = {
            "x": np.ascontiguousarray(x[c * BPC:(c + 1) * BPC]),
            "xt": np.ascontiguousarray(xt_host[c * BPC:(c + 1) * BPC]),

        }
        if use_gb:
            m["gb"] = np.ascontiguousarray(np.stack([g, be]))
        if use_bo:
            m["bo"] = bo
        in_maps.append(m)

    res = run_bass_kernel_spmd(nc, in_maps, core_ids=list(range(NCORES)))
    return np.concatenate([r["y"] for r in res.results], axis=0)


if __name__ == "__main__":
    rng = np.random.default_rng(0)
    inputs = {
        "featureVec": rng.standard_normal((B, F, DIN), dtype=np.float32),
        "Wqkv": (rng.standard_normal((H, 3, DIN, DOUT), dtype=np.float32)
                 / np.sqrt(DIN).astype(np.float32)),
        "Wo": (rng.standard_normal((H * DOUT, DIN), dtype=np.float32)
               / np.sqrt(H * DOUT).astype(np.float32)),
        "bo": np.zeros(DIN, np.float32),
        "ln_gamma": np.ones(DIN, np.float32),
        "ln_beta": np.zeros(DIN, np.float32),
    }
    out = kernel(**inputs)
    print(out.shape, out.dtype, float(np.abs(out).max()))
